# revision 1
# baseline (speedup 1.0000x reference)
"""Masked ragged-attention TRN2 kernel (nn_AttentionBase, B=16 Q=K=D=1024 fp32).

Sharding: data-parallel over batch, 2 batches per NeuronCore, 8 cores.
Per core (uniform SPMD program, masking driven purely by input data):
  scores = Q @ K^T          float32r matmuls, contraction d on partitions
  scores += ones(q) x biasrow(k)   rank-1 matmul; biasrow = 0 / -1e30 per key
  softmax along k (free axis): negated reduce_max -> exp(x - max) with fused
  row-sum on ScalarE -> reciprocal; rows q >= query_len zeroed via the
  per-partition output scale
  out = softmax @ V         PE-transposed weights, float32r matmuls

float32r rounds operands to 12 significant bits but runs at full PE rate.
QK_TERMS=3 uses a Dekker hi/lo split of Q and K (host-side) and three
accumulated fp32r matmuls for a near-fp32-exact score matrix at 3x QK cost.

Host packs Q^T/K^T pre-transposed AND pre-tiled so each SBUF tile is one
large DMA: qt[b][m][d][128][128], kt[b][n2][d][128][512], v[b][d][128][1024].
"""

import sys

sys.path.insert(0, "/opt/trn_rl_repo")

import numpy as np

P = 128
B_PER_CORE = 2
N_CORES = 8
SEQ = 1024
D = 1024
NCH = SEQ // P  # 8 chunks along any 1024 dim
NEG = np.float32(-1e30)

QK_TERMS = 3  # 1 = single fp32r pass, 3 = hi/lo split (near-exact)

_CACHE = {}


def _round_f32r(x):
    """Round fp32 array to float32r precision (11 stored mantissa bits,
    round-to-nearest-even), matching the hardware's in-flight DMA rounding."""
    u = x.view(np.uint32).astype(np.uint64)
    drop = 12  # 23 - 11
    half = np.uint64((1 << (drop - 1)) - 1)
    lsb = (u >> np.uint64(drop)) & np.uint64(1)
    u = (u + half + lsb) >> np.uint64(drop) << np.uint64(drop)
    return u.astype(np.uint32).view(np.float32)


def _build_nc():
    import concourse.bass as bass  # noqa: F401
    import concourse.mybir as mybir
    import concourse.tile as tile
    from concourse import bacc
    from concourse.masks import make_identity

    f32 = mybir.dt.float32
    f32r = mybir.dt.float32r
    X = mybir.AxisListType.X
    Exp = mybir.ActivationFunctionType.Exp

    q_names = ["qt"] if QK_TERMS == 1 else ["qt", "qtl"]
    k_names = ["kt"] if QK_TERMS == 1 else ["kt", "ktl"]

    nc = bacc.Bacc("TRN2", target_bir_lowering=False, debug=False)
    # QK_TERMS == 3 ships ONE fp32 copy of Q^T/K^T; the Dekker hi/lo split is
    # computed on device (DVE round-to-f32r + exact subtract), halving the
    # Q/K input DMA volume.
    q_in_dt = f32r if QK_TERMS == 1 else f32
    q_dram = nc.dram_tensor(
        "qt", [B_PER_CORE, NCH, NCH, P, P], q_in_dt, kind="ExternalInput"
    )  # [b, m, d, p, c]
    k_dram = nc.dram_tensor(
        "kt", [B_PER_CORE, 2, NCH, P, 512], q_in_dt, kind="ExternalInput"
    )  # [b, n2, d, p, c]
    v_d = nc.dram_tensor("v", [B_PER_CORE, NCH, P, D], f32r, kind="ExternalInput")
    bias_d = nc.dram_tensor("bias", [B_PER_CORE, SEQ], f32r, kind="ExternalInput")
    qmask_d = nc.dram_tensor("qmask", [B_PER_CORE, SEQ], f32, kind="ExternalInput")
    out_d = nc.dram_tensor("out", [B_PER_CORE, SEQ, D], f32, kind="ExternalOutput")

    with tile.TileContext(nc) as tc:
        with (
            tc.tile_pool(name="const", bufs=1) as const_pool,
            tc.tile_pool(name="qk", bufs=1) as qk_pool,
            tc.tile_pool(name="v", bufs=1) as v_pool,
            tc.tile_pool(name="work", bufs=2) as work,
            tc.tile_pool(name="wpool", bufs=2) as wpool,
            tc.tile_pool(name="stat", bufs=6) as stat,
            tc.tile_pool(name="stage", bufs=2) as stage,
            tc.tile_pool(name="qstage", bufs=1) as qstage_pool,
            tc.tile_pool(name="misc", bufs=2) as misc,
            tc.tile_pool(name="ps_s", bufs=3, space="PSUM") as ps_s,
            tc.tile_pool(name="ps_t", bufs=3, space="PSUM") as ps_t,
            tc.tile_pool(name="ps_o", bufs=1, space="PSUM") as ps_o,
        ):
            identity_f32 = const_pool.tile([P, P], f32, tag="ident32")
            make_identity(nc, identity_f32)
            identity = const_pool.tile([P, P], f32r, tag="ident")
            nc.vector.tensor_copy(identity[:], identity_f32[:])
            ones_f32 = const_pool.tile([1, P], f32, tag="ones32")
            nc.gpsimd.memset(ones_f32[:], 1.0)
            ones = const_pool.tile([1, P], f32r, tag="ones")
            nc.vector.tensor_copy(ones[:], ones_f32[:])

            for b in range(B_PER_CORE):
                # SBUF tiles: per (tensor, m) Q tiles [P, d, P]; per
                # (tensor, n2, half) K tiles [P, d/2, 512], one DMA per tile.
                # SP carries the main-term loads, ACT the hi/lo extras.
                qt_t = {
                    (n, m): qk_pool.tile([P, NCH, P], f32r, tag=f"{n}{m}", name=f"{n}{m}")
                    for n in q_names
                    for m in range(NCH)
                }
                # ALL K tiles are quartered ([P, 2, 512]) so the first QK
                # chain starts after a 0.5MB transfer and the fp32 staging
                # tile for the on-device split stays small.
                kt_t = {}
                for n in k_names:
                    for n2 in range(2):
                        for h in range(4):
                            kt_t[(n, n2, h)] = qk_pool.tile(
                                [P, 2, 512], f32r, tag=f"{n}{n2}{h}", name=f"{n}{n2}{h}"
                            )

                def split_hi_lo(full, hi_ap, lo_ap):
                    # hi = round-to-f32r(full); lo = full - hi (exact, and the
                    # residual fits f32r so the output rounding is lossless).
                    nc.vector.tensor_copy(hi_ap, full)
                    nc.vector.tensor_tensor(
                        lo_ap, full, hi_ap.bitcast(f32), mybir.AluOpType.subtract
                    )

                def load_q(m, engq=None):
                    engq = engq or nc.scalar
                    if QK_TERMS == 3:
                        st = qstage_pool.tile([P, NCH, P], f32, tag="qstage", name="qstage")
                        engq.dma_start(
                            st[:], q_dram.ap()[b, m].rearrange("d p c -> p d c")
                        )
                        split_hi_lo(st[:], qt_t[("qt", m)][:], qt_t[("qtl", m)][:])
                    else:
                        engq.dma_start(
                            qt_t[("qt", m)][:],
                            q_dram.ap()[b, m].rearrange("d p c -> p d c"),
                        )

                def load_k(n2, h, direct_hi=False):
                    src_ap = k_dram.ap()[b, n2, h * 2 : (h + 1) * 2].rearrange(
                        "d p c -> p d c"
                    )
                    hi = kt_t[("kt", n2, h)][:]
                    if QK_TERMS == 3:
                        st = stage.tile([P, 2, 512], f32, tag="kstage", name="kstage")
                        nc.sync.dma_start(st[:], src_ap)
                        split_hi_lo(st[:], hi, kt_t[("ktl", n2, h)][:])
                    else:
                        nc.sync.dma_start(hi, src_ap)

                # ramp-critical order: everything the m0 score tile needs first
                load_q(0)
                for h in range(4):
                    load_k(0, h)
                for h in range(4):
                    load_k(1, h)
                brow = misc.tile([1, SEQ], f32r, tag="brow")
                nc.gpsimd.dma_start(brow[:], bias_d.ap()[b : b + 1, :])
                qm = stat.tile([P, NCH], f32, tag="qm")
                nc.gpsimd.dma_start(qm[:], qmask_d.ap()[b].rearrange("(t p) -> p t", p=P))
                for m in range(1, 3):
                    load_q(m)
                vc = []
                for d in range(NCH):
                    t = v_pool.tile([P, D], f32r, tag=f"v{d}", name=f"v{d}")
                    nc.sync.dma_start(t[:], v_d.ap()[b, d])
                    vc.append(t)
                for m in range(3, NCH):
                    load_q(m)

                if QK_TERMS == 1:
                    mm_pairs = [("qt", "kt")]
                else:
                    mm_pairs = [("qt", "kt"), ("qt", "ktl"), ("qtl", "kt")]

                stageb = {}

                def emit_stage_a(m):
                    nm2 = stat.tile([P, 2], f32, tag="nm2", name="nm2")
                    negmax = stat.tile([P, 1], f32, tag="negmax", name="negmax")
                    w_sb = wpool.tile([P, SEQ], f32r, tag="w", name="w")
                    rs = stat.tile([P, 2], f32, tag="rs", name="rs")
                    pss = []
                    for n2 in range(2):
                        ps = ps_s.tile([P, 512], f32, tag="s", name=f"s{n2}")
                        first = True
                        for qn, kn in mm_pairs:
                            for d in range(NCH):
                                nc.tensor.matmul(
                                    ps[:],
                                    qt_t[(qn, m)][:, d],
                                    kt_t[(kn, n2, d // 2)][:, d % 2],
                                    start=first,
                                    stop=False,
                                )
                                first = False
                        nc.tensor.matmul(
                            ps[:],
                            ones[:],
                            brow[:, n2 * 512 : (n2 + 1) * 512],
                            start=False,
                            stop=True,
                        )
                        nc.vector.reduce_max(
                            nm2[:, n2 : n2 + 1], ps[:], axis=X, negate=True
                        )
                        pss.append(ps)
                    nc.vector.tensor_tensor(
                        negmax[:], nm2[:, 0:1], nm2[:, 1:2], mybir.AluOpType.min
                    )
                    for n2 in range(2):
                        nc.scalar.activation(
                            w_sb[:, n2 * 512 : (n2 + 1) * 512],
                            pss[n2][:],
                            Exp,
                            bias=negmax[:],
                            accum_out=rs[:, n2 : n2 + 1],
                        )
                    rsum = stat.tile([P, 1], f32, tag="rsum", name="rsum")
                    nc.vector.tensor_tensor(
                        rsum[:], rs[:, 0:1], rs[:, 1:2], mybir.AluOpType.add
                    )
                    rcp = stat.tile([P, 1], f32, tag="rcp", name="rcp")
                    nc.vector.reciprocal(rcp[:], rsum[:])
                    scal = stat.tile([P, 1], f32, tag="scal", name="scal")
                    nc.vector.tensor_tensor(
                        scal[:], rcp[:], qm[:, m : m + 1], mybir.AluOpType.mult
                    )
                    stageb[m] = (w_sb, scal)

                def emit_stage_b(m):
                    w_sb, scal = stageb.pop(m)
                    wt = []
                    for j in range(NCH):
                        pst = ps_t.tile([P, P], f32r, tag="pst", name="pst")
                        nc.tensor.transpose(
                            pst[:], w_sb[:, j * P : (j + 1) * P], identity[:]
                        )
                        wtj = work.tile([P, P], f32r, tag=f"wt{j}", name=f"wt{j}")
                        nc.any.tensor_copy(wtj[:], pst[:])
                        wt.append(wtj)

                    out_sb = work.tile([P, D], f32, tag="outsb")
                    for n2 in range(2):
                        po = ps_o.tile([P, 512], f32, tag=f"o{n2}", name=f"o{n2}")
                        for j in range(NCH):
                            nc.tensor.matmul(
                                po[:],
                                wt[j][:],
                                vc[j][:, n2 * 512 : (n2 + 1) * 512],
                                start=(j == 0),
                                stop=(j == NCH - 1),
                            )
                        nc.any.tensor_scalar_mul(
                            out_sb[:, n2 * 512 : (n2 + 1) * 512], po[:], scal[:]
                        )
                    # the very last store goes via HWDGE (lower latency than
                    # SWDGE) to shorten the kernel-tail drain
                    out_eng = (
                        nc.sync if (b == B_PER_CORE - 1 and m == NCH - 1) else nc.gpsimd
                    )
                    out_eng.dma_start(out_d.ap()[b, m * P : (m + 1) * P, :], out_sb[:])

                for m in range(NCH + 1):
                    if m < NCH:
                        emit_stage_a(m)
                    if m >= 1:
                        emit_stage_b(m - 1)
    nc.compile()
    return nc


def _get_nc():
    if "nc" not in _CACHE:
        _CACHE["nc"] = _build_nc()
    return _CACHE["nc"]


def _q_layout(qT):
    """[d, q] transposed matrix -> [m, d, P, P] host layout."""
    # qt[m, d, p, c] = qT[d*P+p, m*P+c]
    return np.ascontiguousarray(qT.reshape(NCH, P, NCH, P).transpose(2, 0, 1, 3))


def _k_layout(kT):
    """[d, k] transposed matrix -> [n2, d, P, 512] host layout."""
    return np.ascontiguousarray(kT.reshape(NCH, P, 2, 512).transpose(2, 0, 1, 3))


def _prep_in_maps(queries, keys, values, query_lens, key_lens, order):
    """Build per-core input maps. order[c] = list of batch indices for core c."""
    kidx = np.arange(SEQ)
    in_maps = []
    for c in range(N_CORES):
        bs = order[c]
        m = {
            "v": np.empty((B_PER_CORE, NCH, P, D), np.float32),
            "bias": np.empty((B_PER_CORE, SEQ), np.float32),
            "qmask": np.empty((B_PER_CORE, SEQ), np.float32),
            "qt": np.empty((B_PER_CORE, NCH, NCH, P, P), np.float32),
            "kt": np.empty((B_PER_CORE, 2, NCH, P, 512), np.float32),
        }
        for i, b in enumerate(bs):
            qT = np.ascontiguousarray(queries[b].T)
            kT = np.ascontiguousarray(keys[b].T)
            m["qt"][i] = _q_layout(qT)
            m["kt"][i] = _k_layout(kT)
            m["v"][i] = values[b].reshape(NCH, P, D)
            m["bias"][i] = np.where(kidx < key_lens[b], np.float32(0.0), NEG)
            m["qmask"][i] = (kidx < query_lens[b]).astype(np.float32)
        in_maps.append(m)
    return in_maps


def _run(inputs, trace=False, trace_kwargs=None):
    from concourse.bass_utils import run_bass_kernel_spmd

    queries = np.asarray(inputs["queries"], dtype=np.float32)
    keys = np.asarray(inputs["keys"], dtype=np.float32)
    values = np.asarray(inputs["values"], dtype=np.float32)
    query_lens = np.asarray(inputs["query_lens"]).astype(np.int64)
    key_lens = np.asarray(inputs["key_lens"]).astype(np.int64)
    B = queries.shape[0]
    assert B == N_CORES * B_PER_CORE

    order = [list(range(c * B_PER_CORE, (c + 1) * B_PER_CORE)) for c in range(N_CORES)]
    in_maps = _prep_in_maps(queries, keys, values, query_lens, key_lens, order)

    nc = _get_nc()
    kwargs = {}
    if trace:
        kwargs["trace"] = True
        if trace_kwargs:
            kwargs.update(trace_kwargs)
    try:
        res = run_bass_kernel_spmd(nc, in_maps, core_ids=list(range(N_CORES)), **kwargs)
    except Exception:
        # transient device wedges (NRT_EXEC_UNIT_UNRECOVERABLE) usually clear
        # on the next attempt
        import time

        time.sleep(5)
        res = run_bass_kernel_spmd(nc, in_maps, core_ids=list(range(N_CORES)), **kwargs)

    out = np.empty((B, SEQ, D), np.float32)
    for c in range(N_CORES):
        o = res.results[c]["out"]
        for i, b in enumerate(order[c]):
            out[b] = o[i]
    return out, res


def kernel(**inputs) -> np.ndarray:
    out, _ = _run(inputs, trace=False)
    return out



# revision 2
# speedup vs baseline: 1.6187x; 1.6187x over previous
"""Masked ragged-attention TRN2 kernel (nn_AttentionBase, B=16 Q=K=D=1024 fp32).

Sharding: data-parallel over batch, 2 batches per NeuronCore, 8 cores.
Per core (uniform SPMD program, masking driven purely by input data):
  scores = Q @ K^T          float32r matmuls, contraction d on partitions
  scores += ones(q) x biasrow(k)   rank-1 matmul; biasrow = 0 / -1e30 per key
  softmax along k (free axis): negated reduce_max -> exp(x - max) with fused
  row-sum on ScalarE -> reciprocal; rows q >= query_len zeroed via the
  per-partition output scale
  out = softmax @ V         PE-transposed weights, float32r matmuls

float32r rounds operands to 12 significant bits but runs at full PE rate.
QK_TERMS=3 uses a Dekker hi/lo split of Q and K (host-side) and three
accumulated fp32r matmuls for a near-fp32-exact score matrix at 3x QK cost.

Host packs Q^T/K^T pre-transposed AND pre-tiled so each SBUF tile is one
large DMA: qt[b][m][d][128][128], kt[b][n2][d][128][512], v[b][d][128][1024].
"""

import sys

sys.path.insert(0, "/opt/trn_rl_repo")

import numpy as np

P = 128
B_PER_CORE = 2
N_CORES = 8
SEQ = 1024
D = 1024
NCH = SEQ // P  # 8 chunks along any 1024 dim
NEG = np.float32(-1e30)

QK_TERMS = 1  # 1 = single fp32r pass, 3 = hi/lo split (near-exact)

_CACHE = {}


def _round_f32r(x):
    """Round fp32 array to float32r precision (11 stored mantissa bits,
    round-to-nearest-even), matching the hardware's in-flight DMA rounding."""
    u = x.view(np.uint32).astype(np.uint64)
    drop = 12  # 23 - 11
    half = np.uint64((1 << (drop - 1)) - 1)
    lsb = (u >> np.uint64(drop)) & np.uint64(1)
    u = (u + half + lsb) >> np.uint64(drop) << np.uint64(drop)
    return u.astype(np.uint32).view(np.float32)


def _build_nc():
    import concourse.bass as bass  # noqa: F401
    import concourse.mybir as mybir
    import concourse.tile as tile
    from concourse import bacc
    from concourse.masks import make_identity

    f32 = mybir.dt.float32
    f32r = mybir.dt.float32r
    X = mybir.AxisListType.X
    Exp = mybir.ActivationFunctionType.Exp

    q_names = ["qt"] if QK_TERMS == 1 else ["qt", "qtl"]
    k_names = ["kt"] if QK_TERMS == 1 else ["kt", "ktl"]

    nc = bacc.Bacc("TRN2", target_bir_lowering=False, debug=False)
    # QK_TERMS == 3 ships ONE fp32 copy of Q^T/K^T; the Dekker hi/lo split is
    # computed on device (DVE round-to-f32r + exact subtract), halving the
    # Q/K input DMA volume.
    q_in_dt = f32r if QK_TERMS == 1 else f32
    q_dram = nc.dram_tensor(
        "qt", [B_PER_CORE, NCH, NCH, P, P], q_in_dt, kind="ExternalInput"
    )  # [b, m, d, p, c]
    k_dram = nc.dram_tensor(
        "kt", [B_PER_CORE, 2, NCH, P, 512], q_in_dt, kind="ExternalInput"
    )  # [b, n2, d, p, c]
    v_d = nc.dram_tensor("v", [B_PER_CORE, NCH, P, D], f32r, kind="ExternalInput")
    bias_d = nc.dram_tensor("bias", [B_PER_CORE, SEQ], f32r, kind="ExternalInput")
    qmask_d = nc.dram_tensor("qmask", [B_PER_CORE, SEQ], f32, kind="ExternalInput")
    out_d = nc.dram_tensor("out", [B_PER_CORE, SEQ, D], f32, kind="ExternalOutput")

    with tile.TileContext(nc) as tc:
        with (
            tc.tile_pool(name="const", bufs=1) as const_pool,
            tc.tile_pool(name="qk", bufs=1) as qk_pool,
            tc.tile_pool(name="v", bufs=1) as v_pool,
            tc.tile_pool(name="work", bufs=2) as work,
            tc.tile_pool(name="wpool", bufs=2) as wpool,
            tc.tile_pool(name="stat", bufs=6) as stat,
            tc.tile_pool(name="stage", bufs=2) as stage,
            tc.tile_pool(name="qstage", bufs=1) as qstage_pool,
            tc.tile_pool(name="misc", bufs=2) as misc,
            tc.tile_pool(name="ps_s", bufs=3, space="PSUM") as ps_s,
            tc.tile_pool(name="ps_t", bufs=3, space="PSUM") as ps_t,
            tc.tile_pool(name="ps_o", bufs=1, space="PSUM") as ps_o,
        ):
            identity_f32 = const_pool.tile([P, P], f32, tag="ident32")
            make_identity(nc, identity_f32)
            identity = const_pool.tile([P, P], f32r, tag="ident")
            nc.vector.tensor_copy(identity[:], identity_f32[:])
            ones_f32 = const_pool.tile([1, P], f32, tag="ones32")
            nc.gpsimd.memset(ones_f32[:], 1.0)
            ones = const_pool.tile([1, P], f32r, tag="ones")
            nc.vector.tensor_copy(ones[:], ones_f32[:])

            for b in range(B_PER_CORE):
                # SBUF tiles: per (tensor, m) Q tiles [P, d, P]; per
                # (tensor, n2, half) K tiles [P, d/2, 512], one DMA per tile.
                # SP carries the main-term loads, ACT the hi/lo extras.
                qt_t = {
                    (n, m): qk_pool.tile([P, NCH, P], f32r, tag=f"{n}{m}", name=f"{n}{m}")
                    for n in q_names
                    for m in range(NCH)
                }
                # ALL K tiles are quartered ([P, 2, 512]) so the first QK
                # chain starts after a 0.5MB transfer and the fp32 staging
                # tile for the on-device split stays small.
                kt_t = {}
                for n in k_names:
                    for n2 in range(2):
                        for h in range(4):
                            kt_t[(n, n2, h)] = qk_pool.tile(
                                [P, 2, 512], f32r, tag=f"{n}{n2}{h}", name=f"{n}{n2}{h}"
                            )

                def split_hi_lo(full, hi_ap, lo_ap):
                    # hi = round-to-f32r(full); lo = full - hi (exact, and the
                    # residual fits f32r so the output rounding is lossless).
                    nc.vector.tensor_copy(hi_ap, full)
                    nc.vector.tensor_tensor(
                        lo_ap, full, hi_ap.bitcast(f32), mybir.AluOpType.subtract
                    )

                def load_q(m, engq=None):
                    engq = engq or nc.scalar
                    if QK_TERMS == 3:
                        st = qstage_pool.tile([P, NCH, P], f32, tag="qstage", name="qstage")
                        engq.dma_start(
                            st[:], q_dram.ap()[b, m].rearrange("d p c -> p d c")
                        )
                        split_hi_lo(st[:], qt_t[("qt", m)][:], qt_t[("qtl", m)][:])
                    else:
                        engq.dma_start(
                            qt_t[("qt", m)][:],
                            q_dram.ap()[b, m].rearrange("d p c -> p d c"),
                        )

                def load_k(n2, h, direct_hi=False):
                    src_ap = k_dram.ap()[b, n2, h * 2 : (h + 1) * 2].rearrange(
                        "d p c -> p d c"
                    )
                    hi = kt_t[("kt", n2, h)][:]
                    if QK_TERMS == 3:
                        st = stage.tile([P, 2, 512], f32, tag="kstage", name="kstage")
                        nc.sync.dma_start(st[:], src_ap)
                        split_hi_lo(st[:], hi, kt_t[("ktl", n2, h)][:])
                    else:
                        nc.sync.dma_start(hi, src_ap)

                # ramp-critical order: everything the m0 score tile needs first
                load_q(0)
                for h in range(4):
                    load_k(0, h)
                for h in range(4):
                    load_k(1, h)
                brow = misc.tile([1, SEQ], f32r, tag="brow")
                nc.gpsimd.dma_start(brow[:], bias_d.ap()[b : b + 1, :])
                qm = stat.tile([P, NCH], f32, tag="qm")
                nc.gpsimd.dma_start(qm[:], qmask_d.ap()[b].rearrange("(t p) -> p t", p=P))
                for m in range(1, 3):
                    load_q(m)
                vc = []
                for d in range(NCH):
                    t = v_pool.tile([P, D], f32r, tag=f"v{d}", name=f"v{d}")
                    nc.sync.dma_start(t[:], v_d.ap()[b, d])
                    vc.append(t)
                for m in range(3, NCH):
                    load_q(m)

                if QK_TERMS == 1:
                    mm_pairs = [("qt", "kt")]
                else:
                    mm_pairs = [("qt", "kt"), ("qt", "ktl"), ("qtl", "kt")]

                stageb = {}

                def emit_stage_a(m):
                    nm2 = stat.tile([P, 2], f32, tag="nm2", name="nm2")
                    negmax = stat.tile([P, 1], f32, tag="negmax", name="negmax")
                    w_sb = wpool.tile([P, SEQ], f32r, tag="w", name="w")
                    rs = stat.tile([P, 2], f32, tag="rs", name="rs")
                    pss = []
                    for n2 in range(2):
                        ps = ps_s.tile([P, 512], f32, tag="s", name=f"s{n2}")
                        first = True
                        for qn, kn in mm_pairs:
                            for d in range(NCH):
                                nc.tensor.matmul(
                                    ps[:],
                                    qt_t[(qn, m)][:, d],
                                    kt_t[(kn, n2, d // 2)][:, d % 2],
                                    start=first,
                                    stop=False,
                                )
                                first = False
                        nc.tensor.matmul(
                            ps[:],
                            ones[:],
                            brow[:, n2 * 512 : (n2 + 1) * 512],
                            start=False,
                            stop=True,
                        )
                        nc.vector.reduce_max(
                            nm2[:, n2 : n2 + 1], ps[:], axis=X, negate=True
                        )
                        pss.append(ps)
                    nc.vector.tensor_tensor(
                        negmax[:], nm2[:, 0:1], nm2[:, 1:2], mybir.AluOpType.min
                    )
                    for n2 in range(2):
                        nc.scalar.activation(
                            w_sb[:, n2 * 512 : (n2 + 1) * 512],
                            pss[n2][:],
                            Exp,
                            bias=negmax[:],
                            accum_out=rs[:, n2 : n2 + 1],
                        )
                    rsum = stat.tile([P, 1], f32, tag="rsum", name="rsum")
                    nc.vector.tensor_tensor(
                        rsum[:], rs[:, 0:1], rs[:, 1:2], mybir.AluOpType.add
                    )
                    rcp = stat.tile([P, 1], f32, tag="rcp", name="rcp")
                    nc.vector.reciprocal(rcp[:], rsum[:])
                    scal = stat.tile([P, 1], f32, tag="scal", name="scal")
                    nc.vector.tensor_tensor(
                        scal[:], rcp[:], qm[:, m : m + 1], mybir.AluOpType.mult
                    )
                    stageb[m] = (w_sb, scal)

                def emit_stage_b(m):
                    w_sb, scal = stageb.pop(m)
                    wt = []
                    for j in range(NCH):
                        pst = ps_t.tile([P, P], f32r, tag="pst", name="pst")
                        nc.tensor.transpose(
                            pst[:], w_sb[:, j * P : (j + 1) * P], identity[:]
                        )
                        wtj = work.tile([P, P], f32r, tag=f"wt{j}", name=f"wt{j}")
                        nc.any.tensor_copy(wtj[:], pst[:])
                        wt.append(wtj)

                    out_sb = work.tile([P, D], f32, tag="outsb")
                    for n2 in range(2):
                        po = ps_o.tile([P, 512], f32, tag=f"o{n2}", name=f"o{n2}")
                        for j in range(NCH):
                            nc.tensor.matmul(
                                po[:],
                                wt[j][:],
                                vc[j][:, n2 * 512 : (n2 + 1) * 512],
                                start=(j == 0),
                                stop=(j == NCH - 1),
                            )
                        nc.any.tensor_scalar_mul(
                            out_sb[:, n2 * 512 : (n2 + 1) * 512], po[:], scal[:]
                        )
                    # the very last store goes via HWDGE (lower latency than
                    # SWDGE) to shorten the kernel-tail drain
                    out_eng = (
                        nc.sync if (b == B_PER_CORE - 1 and m == NCH - 1) else nc.gpsimd
                    )
                    out_eng.dma_start(out_d.ap()[b, m * P : (m + 1) * P, :], out_sb[:])

                for m in range(NCH + 1):
                    if m < NCH:
                        emit_stage_a(m)
                    if m >= 1:
                        emit_stage_b(m - 1)
    nc.compile()
    return nc


def _get_nc():
    if "nc" not in _CACHE:
        _CACHE["nc"] = _build_nc()
    return _CACHE["nc"]


def _q_layout(qT):
    """[d, q] transposed matrix -> [m, d, P, P] host layout."""
    # qt[m, d, p, c] = qT[d*P+p, m*P+c]
    return np.ascontiguousarray(qT.reshape(NCH, P, NCH, P).transpose(2, 0, 1, 3))


def _k_layout(kT):
    """[d, k] transposed matrix -> [n2, d, P, 512] host layout."""
    return np.ascontiguousarray(kT.reshape(NCH, P, 2, 512).transpose(2, 0, 1, 3))


def _prep_in_maps(queries, keys, values, query_lens, key_lens, order):
    """Build per-core input maps. order[c] = list of batch indices for core c."""
    kidx = np.arange(SEQ)
    in_maps = []
    for c in range(N_CORES):
        bs = order[c]
        m = {
            "v": np.empty((B_PER_CORE, NCH, P, D), np.float32),
            "bias": np.empty((B_PER_CORE, SEQ), np.float32),
            "qmask": np.empty((B_PER_CORE, SEQ), np.float32),
            "qt": np.empty((B_PER_CORE, NCH, NCH, P, P), np.float32),
            "kt": np.empty((B_PER_CORE, 2, NCH, P, 512), np.float32),
        }
        for i, b in enumerate(bs):
            qT = np.ascontiguousarray(queries[b].T)
            kT = np.ascontiguousarray(keys[b].T)
            m["qt"][i] = _q_layout(qT)
            m["kt"][i] = _k_layout(kT)
            m["v"][i] = values[b].reshape(NCH, P, D)
            m["bias"][i] = np.where(kidx < key_lens[b], np.float32(0.0), NEG)
            m["qmask"][i] = (kidx < query_lens[b]).astype(np.float32)
        in_maps.append(m)
    return in_maps


def _run(inputs, trace=False, trace_kwargs=None):
    from concourse.bass_utils import run_bass_kernel_spmd

    queries = np.asarray(inputs["queries"], dtype=np.float32)
    keys = np.asarray(inputs["keys"], dtype=np.float32)
    values = np.asarray(inputs["values"], dtype=np.float32)
    query_lens = np.asarray(inputs["query_lens"]).astype(np.int64)
    key_lens = np.asarray(inputs["key_lens"]).astype(np.int64)
    B = queries.shape[0]
    assert B == N_CORES * B_PER_CORE

    order = [list(range(c * B_PER_CORE, (c + 1) * B_PER_CORE)) for c in range(N_CORES)]
    in_maps = _prep_in_maps(queries, keys, values, query_lens, key_lens, order)

    nc = _get_nc()
    kwargs = {}
    if trace:
        kwargs["trace"] = True
        if trace_kwargs:
            kwargs.update(trace_kwargs)
    try:
        res = run_bass_kernel_spmd(nc, in_maps, core_ids=list(range(N_CORES)), **kwargs)
    except Exception:
        # transient device wedges (NRT_EXEC_UNIT_UNRECOVERABLE) usually clear
        # on the next attempt
        import time

        time.sleep(5)
        res = run_bass_kernel_spmd(nc, in_maps, core_ids=list(range(N_CORES)), **kwargs)

    out = np.empty((B, SEQ, D), np.float32)
    for c in range(N_CORES):
        o = res.results[c]["out"]
        for i, b in enumerate(order[c]):
            out[b] = o[i]
    return out, res


def kernel(**inputs) -> np.ndarray:
    out, _ = _run(inputs, trace=False)
    return out



# revision 8
# speedup vs baseline: 2.1263x; 1.3136x over previous
"""Ragged masked-attention TRN2 kernel (nn_AttentionBase, B=16 Q=K=D=1024 fp32).

Sharding: data-parallel over batch, 8 cores, one SPMD program. The program
is a static schedule of per-q-tile "tasks" computed at runtime from the
actual query_lens/key_lens: rows beyond query_len and keys beyond key_len
contribute nothing to the output (softmax weight 0 / output row 0), so the
schedule only covers q-tiles < ceil(q_len/128) and per-task k-windows of
ceil(key_len/128) chunks (padded to >=2 for full-rate fp32r matmuls).

Structure: 2 phases per core (one resident K/V set each, double-buffered);
each phase has T tasks with static per-task k-window sizes taken from the
phase "profile" (elementwise max over the 8 batches assigned to the phase).
Host packs Q^T tiles / K^T / V / bias / qmask per (core, phase, task) and
scatters the bf16 task outputs back to the full fp32 output.

Numerics: scores in fp32r (PE full rate), softmax stats in fp32,
exp-weights / V / output in bf16. Measured rel err ~7e-3 vs 2e-2 gate.
"""

import sys

sys.path.insert(0, "/opt/trn_rl_repo")

from itertools import combinations

import numpy as np
import ml_dtypes

P = 128
N_CORES = 8
SEQ = 1024
D = 1024
NCH = SEQ // P  # 8 chunks of 128 along any 1024 dim
NEG = np.float32(-1e30)

_CACHE = {}


# ---------------------------------------------------------------- schedule
#
# Schedule model: each phase loads one K/V buffer of W k-chunks, laid out as
# disjoint "slots" (offset, width). A slot has a task-window profile; every
# core runs every task. A core may host at most one batch per slot (its K/V
# at the slot offset); a batch may be split across several cores' cells of
# the same slot (each cell computes a subset of its q-tiles against the
# batch's full K). Cost per core is static: sum over tasks of the window
# size. Search minimizes an estimated wall time (PE vs DMA roofline).

W_MAX = 10  # max K chunks resident per phase (SBUF budget)


def _slot_profile(members, qm, km):
    """members: {batch: ncells}. Task profile (desc): profile[t] = max km
    over members whose per-cell tile count exceeds t."""
    tiles = {b: -(-qm[b] // n) for b, n in members.items()}
    L = max(tiles.values())
    return [max(km[b] for b in members if tiles[b] > t) for t in range(L)]


def _eval_state(state, qm, km):
    units = tasks = 0
    Ws = []
    for ph in state:
        W = 0
        for s in ph:
            if not s["m"]:
                continue
            if sum(s["m"].values()) > 8 or any(km[b] > s["w"] for b in s["m"]):
                return None
            prof = _slot_profile(s["m"], qm, km)
            units += sum(prof)
            tasks += len(prof)
            W += s["w"]
        if W > W_MAX or W == 0:
            return None
        Ws.append(W)
    pe = units * 960 + tasks * 250
    mb = sum(Ws) * 0.75 + tasks * 0.75  # K f32r + V bf16, Q f32r + out bf16
    dma = mb * 2900
    startup = min(Ws) * 0.5 * 2900 + 4000
    return max(pe, dma) + startup + 6000, units, tasks


def _search_schedule(qm, km, iters=40000):
    """Simulated-annealing search over slot layouts/assignments."""
    import math
    import random

    B = len(qm)
    rng = random.Random(0)

    # init: best 8/8 partition, one slot per phase (variant-c equivalent)
    def profile_cost(group):
        T = max(qm[b] for b in group)
        return sum(max(km[b] for b in group if qm[b] > t) for t in range(T))

    best_part = None
    allb = list(range(B))
    for g0 in combinations(allb, B // 2):
        g1 = tuple(b for b in allb if b not in g0)
        c = profile_cost(g0) + profile_cost(g1)
        if best_part is None or c < best_part[0]:
            best_part = (c, g0, g1)
    _, g0, g1 = best_part
    state = [
        [{"w": max(km[b] for b in g), "m": {b: 1 for b in g}}] for g in (g0, g1)
    ]

    cur = _eval_state(state, qm, km)
    assert cur is not None
    best = (cur[0], [[{"w": s["w"], "m": dict(s["m"])} for s in ph] for ph in state])

    def copy_state(st):
        return [[{"w": s["w"], "m": dict(s["m"])} for s in ph] for ph in st]

    temp0 = 3000.0
    for it in range(iters):
        temp = temp0 * (1.0 - it / iters) + 1.0
        cand = copy_state(state)
        # pick a random batch placement
        locs = [
            (pi, si, b)
            for pi, ph in enumerate(cand)
            for si, s in enumerate(ph)
            for b in s["m"]
        ]
        pi, si, b = locs[rng.randrange(len(locs))]
        mv = rng.random()
        if mv < 0.45:
            # relocate batch to another slot (possibly new)
            n = cand[pi][si]["m"].pop(b)
            tpi = rng.randrange(len(cand))
            tph = cand[tpi]
            choices = [s for s in tph if s["w"] >= km[b]] + ["new"]
            tgt = choices[rng.randrange(len(choices))]
            if tgt == "new":
                tph.append({"w": km[b], "m": {b: n}})
            else:
                tgt["m"][b] = tgt["m"].get(b, 0) + n if False else n
        elif mv < 0.8:
            # change split factor
            n = cand[pi][si]["m"][b]
            cand[pi][si]["m"][b] = max(1, n + rng.choice([-1, 1]))
        else:
            # change slot width
            s = cand[pi][si]
            s["w"] = max(max(km[x] for x in s["m"]), s["w"] + rng.choice([-1, 1]))
        for ph in cand:
            ph[:] = [s for s in ph if s["m"]]
        if not all(ph for ph in cand):
            continue
        r = _eval_state(cand, qm, km)
        if r is None:
            continue
        if r[0] <= cur[0] or rng.random() < math.exp((cur[0] - r[0]) / temp):
            state, cur = cand, r
            if r[0] < best[0]:
                best = (r[0], copy_state(state))
    return best[1]


def _make_schedule(query_lens, key_lens):
    B = len(query_lens)
    qm = [max(1, -(-int(q) // P)) for q in query_lens]
    km = [max(2, -(-int(k) // P)) for k in key_lens]  # >=2 keeps matmul F>=256

    state = _search_schedule(qm, km)
    # order phases smallest-W first (startup), slots narrow-first
    state.sort(key=lambda ph: sum(sum(_slot_profile(s["m"], qm, km)) for s in ph))

    phases = []
    for ph_slots in state:
        ph_slots.sort(key=lambda s: s["w"])
        tasks = []
        assign = [[] for _ in range(N_CORES)]
        off = 0
        for s in ph_slots:
            prof = _slot_profile(s["m"], qm, km)
            base = len(tasks)
            # ascending task order within the slot (small windows first)
            asc = prof[::-1]
            tasks.extend((off, kw) for kw in asc)
            L = len(prof)
            core = 0
            for b, n in sorted(s["m"].items()):
                # distribute qm[b] tiles over n cells (cores)
                tiles = qm[b]
                per = -(-tiles // n)
                done = 0
                for cell in range(n):
                    cnt = min(per, tiles - done)
                    if cnt <= 0:
                        break
                    # cell uses the cnt largest tasks = last cnt in asc order
                    tlist = [
                        (base + L - cnt + j, done + j) for j in range(cnt)
                    ]
                    assign[core].append((b, off, tlist))
                    core += 1
                    done += cnt
            off += s["w"]
        phases.append({"w": off, "tasks": tasks, "assign": assign})
    skeleton = tuple((ph["w"], tuple(ph["tasks"])) for ph in phases)
    return phases, skeleton, qm, km


# ---------------------------------------------------------------- program


def _build_nc(skeleton):
    import concourse.bass as bass  # noqa: F401
    import concourse.mybir as mybir
    import concourse.tile as tile
    from concourse import bacc
    from concourse.masks import make_identity

    f32 = mybir.dt.float32
    f32r = mybir.dt.float32r
    bf16 = mybir.dt.bfloat16
    X = mybir.AxisListType.X
    Exp = mybir.ActivationFunctionType.Exp

    phases = [{"w": w, "tasks": list(tasks)} for w, tasks in skeleton]
    wmax = max(ph["w"] for ph in phases)
    tmax = max(len(ph["tasks"]) for ph in phases)

    nc = bacc.Bacc("TRN2", target_bir_lowering=False, debug=False)

    for p, ph in enumerate(phases):
        w, T = ph["w"], len(ph["tasks"])
        ph["k_d"] = nc.dram_tensor(f"k{p}", [NCH, P, w * P], f32r, kind="ExternalInput")
        ph["v_d"] = nc.dram_tensor(f"v{p}", [w, P, D], bf16, kind="ExternalInput")
        ph["q_d"] = nc.dram_tensor(f"q{p}", [T, NCH, P, P], f32r, kind="ExternalInput")
        ph["bias_d"] = nc.dram_tensor(f"bias{p}", [1, T, w * P], f32r, kind="ExternalInput")
        ph["qm_d"] = nc.dram_tensor(f"qm{p}", [P, T], f32, kind="ExternalInput")
        ph["out_d"] = nc.dram_tensor(f"out{p}", [T, P, D], bf16, kind="ExternalOutput")

    def score_chunks(kw):
        """Split kw*128 score columns into PSUM chunks, each <=512 and
        (for fp32r full rate) >=256 columns."""
        n = kw * P
        if n <= 512:
            return [(0, n)]
        if n <= 896:
            h = (kw // 2) * P
            return [(0, h), (h, n - h)]
        return [(0, 512), (512, n - 512)]

    with tile.TileContext(nc) as tc:
        with (
            tc.tile_pool(name="const", bufs=1) as const_pool,
            tc.tile_pool(name="kv", bufs=1) as kv_pool,
            tc.tile_pool(name="qs", bufs=3) as qs_pool,
            tc.tile_pool(name="w", bufs=2) as w_pool,
            tc.tile_pool(name="wt", bufs=2) as wt_pool,
            tc.tile_pool(name="ob", bufs=2) as ob_pool,
            tc.tile_pool(name="stat", bufs=6) as stat,
            tc.tile_pool(name="ps_s", bufs=3, space="PSUM") as ps_s,
            tc.tile_pool(name="ps_t", bufs=3, space="PSUM") as ps_t,
            tc.tile_pool(name="ps_o", bufs=1, space="PSUM") as ps_o,
        ):
            identity_f32 = const_pool.tile([P, P], f32, tag="ident32")
            make_identity(nc, identity_f32)
            identity = const_pool.tile([P, P], bf16, tag="ident")
            nc.vector.tensor_copy(identity[:], identity_f32[:])
            ones_f32 = const_pool.tile([1, P], f32, tag="ones32")
            nc.gpsimd.memset(ones_f32[:], 1.0)
            ones = const_pool.tile([1, P], f32r, tag="ones")
            nc.vector.tensor_copy(ones[:], ones_f32[:])

            # per-phase-parity K/V/bias/qmask tiles (double buffer)
            kt = {}
            vt = {}
            bt = {}
            qmt = {}
            for par in range(2):
                kt[par] = kv_pool.tile(
                    [P, NCH, wmax * P], f32r, tag=f"k{par}", name=f"k{par}"
                )
                vt[par] = kv_pool.tile([P, wmax, D], bf16, tag=f"v{par}", name=f"v{par}")
                bt[par] = kv_pool.tile(
                    [1, tmax, wmax * P], f32r, tag=f"b{par}", name=f"b{par}"
                )
                qmt[par] = kv_pool.tile([P, tmax], f32, tag=f"m{par}", name=f"m{par}")

            def load_phase(p, skip_v=False):
                ph, par = phases[p], p % 2
                w, T = ph["w"], len(ph["tasks"])
                for d in range(NCH):
                    nc.sync.dma_start(kt[par][:, d, : w * P], ph["k_d"].ap()[d])
                nc.gpsimd.dma_start(bt[par][:, :T, : w * P], ph["bias_d"].ap())
                nc.gpsimd.dma_start(qmt[par][:, :T], ph["qm_d"].ap())
                if not skip_v:
                    load_v(p)

            def load_v(p):
                ph, par = phases[p], p % 2
                w = ph["w"]
                nc.sync.dma_start(
                    vt[par][:, :w], ph["v_d"].ap().rearrange("j p c -> p j c")
                )

            def load_q(p, t):
                ph = phases[p]
                qtile = qs_pool.tile([P, NCH, P], f32r, tag="q", name=f"q{p}_{t}")
                nc.sync.dma_start(
                    qtile[:], ph["q_d"].ap()[t].rearrange("d p c -> p d c")
                )
                return qtile

            stageb = {}

            def emit_stage_a(p, t, qtile):
                ph, par = phases[p], p % 2
                off, kw = ph["tasks"][t]
                chunks = score_chunks(kw)
                w_sb = w_pool.tile([P, wmax * P], bf16, tag="w", name="w")
                nm2 = stat.tile([P, 2], f32, tag="nm2", name="nm2")
                rs = stat.tile([P, 2], f32, tag="rs", name="rs")
                pss = []
                for i, (c0, sz) in enumerate(chunks):
                    ps = ps_s.tile([P, 512], f32, tag="s", name=f"s{i}")
                    for d in range(NCH):
                        nc.tensor.matmul(
                            ps[:, :sz],
                            qtile[:, d],
                            kt[par][:, d, off * P + c0 : off * P + c0 + sz],
                            start=(d == 0),
                            stop=False,
                        )
                    nc.tensor.matmul(
                        ps[:, :sz],
                        ones[:],
                        bt[par][:, t, c0 : c0 + sz],
                        start=False,
                        stop=True,
                    )
                    nc.vector.reduce_max(nm2[:, i : i + 1], ps[:, :sz], axis=X, negate=True)
                    pss.append(ps)
                if len(chunks) == 2:
                    negmax = stat.tile([P, 1], f32, tag="negmax", name="negmax")
                    nc.vector.tensor_tensor(
                        negmax[:], nm2[:, 0:1], nm2[:, 1:2], mybir.AluOpType.min
                    )
                else:
                    negmax = nm2[:, 0:1]
                for i, (c0, sz) in enumerate(chunks):
                    nc.scalar.activation(
                        w_sb[:, c0 : c0 + sz],
                        pss[i][:, :sz],
                        Exp,
                        bias=negmax if len(chunks) == 2 else nm2[:, 0:1],
                        accum_out=rs[:, i : i + 1],
                    )
                if len(chunks) == 2:
                    rsum = stat.tile([P, 1], f32, tag="rsum", name="rsum")
                    nc.vector.tensor_tensor(
                        rsum[:], rs[:, 0:1], rs[:, 1:2], mybir.AluOpType.add
                    )
                else:
                    rsum = rs[:, 0:1]
                rcp = stat.tile([P, 1], f32, tag="rcp", name="rcp")
                nc.vector.reciprocal(rcp[:], rsum)
                scal = stat.tile([P, 1], f32, tag="scal", name="scal")
                nc.vector.tensor_tensor(
                    scal[:], rcp[:], qmt[par][:, t : t + 1], mybir.AluOpType.mult
                )
                stageb[(p, t)] = (w_sb, scal)

            def emit_stage_b(p, t, last):
                ph, par = phases[p], p % 2
                off, kw = ph["tasks"][t]
                w_sb, scal = stageb.pop((p, t))
                wts = []
                for j in range(kw):
                    pst = ps_t.tile([P, P], bf16, tag="pst", name="pst")
                    nc.tensor.transpose(
                        pst[:], w_sb[:, j * P : (j + 1) * P], identity[:]
                    )
                    wtj = wt_pool.tile([P, P], bf16, tag=f"wt{j}", name=f"wt{j}")
                    nc.any.tensor_copy(wtj[:], pst[:])
                    wts.append(wtj)
                out_sb = ob_pool.tile([P, D], bf16, tag="outsb")
                for n2 in range(2):
                    po = ps_o.tile([P, 512], f32, tag=f"o{n2}", name=f"o{n2}")
                    for j in range(kw):
                        nc.tensor.matmul(
                            po[:],
                            wts[j][:],
                            vt[par][:, off + j, n2 * 512 : (n2 + 1) * 512],
                            start=(j == 0),
                            stop=(j == kw - 1),
                        )
                    nc.any.tensor_scalar_mul(
                        out_sb[:, n2 * 512 : (n2 + 1) * 512], po[:], scal[:]
                    )
                out_eng = nc.sync if last else nc.gpsimd
                out_eng.dma_start(ph["out_d"].ap()[t], out_sb[:])

            # flat task list with cross-phase software pipeline
            flat = [(p, t) for p, ph in enumerate(phases) for t in range(len(ph["tasks"]))]
            load_phase(0, skip_v=True)
            qtiles = {}
            for i in range(min(2, len(flat))):
                qtiles[flat[i]] = load_q(*flat[i])
            load_v(0)  # behind K(p0)+Q(0,1) in queue: first QK isn't starved
            for i, (p, t) in enumerate(flat):
                emit_stage_a(p, t, qtiles.pop((p, t)))
                if i + 2 < len(flat):
                    qtiles[flat[i + 2]] = load_q(*flat[i + 2])
                if i == 0 and len(phases) > 1:
                    load_phase(1)
                if i >= 1:
                    pp, tt = flat[i - 1]
                    emit_stage_b(pp, tt, last=False)
            pp, tt = flat[-1]
            emit_stage_b(pp, tt, last=True)
    nc.compile()
    return nc


def _get_nc(skeleton=None):
    if skeleton is None:
        skeleton = _CACHE.get("last_skeleton")
        assert skeleton is not None, "no schedule computed yet"
    if ("nc", skeleton) not in _CACHE:
        _CACHE[("nc", skeleton)] = _build_nc(skeleton)
    _CACHE["last_skeleton"] = skeleton
    return _CACHE[("nc", skeleton)]


# ---------------------------------------------------------------- host side


def _prep_in_maps(phases, qm, km, queries, keys, values, query_lens, key_lens):
    kidx = np.arange(SEQ)
    in_maps = []
    for c in range(N_CORES):
        m = {}
        for p, ph in enumerate(phases):
            w, T = ph["w"], len(ph["tasks"])
            kbuf = np.zeros((NCH, P, w * P), np.float32)
            vbuf = np.zeros((w, P, D), ml_dtypes.bfloat16)
            qbuf = np.zeros((T, NCH, P, P), np.float32)
            bbuf = np.zeros((1, T, w * P), np.float32)
            qmbuf = np.zeros((P, T), np.float32)
            if c < len(ph["assign"]):
                b, tasks = ph["assign"][c]
                kmb, qlb, klb = km[b], int(query_lens[b]), int(key_lens[b])
                kT = keys[b].T.reshape(NCH, P, SEQ)  # [d, p, k]
                kbuf[:, :, : kmb * P] = kT[:, :, : kmb * P]
                vbuf[:kmb] = values[b].reshape(NCH, P, D)[:kmb].astype(ml_dtypes.bfloat16)
                qT = queries[b].T.reshape(NCH, P, NCH, P)  # [d, p, m, c]
                for t, qt in tasks:
                    off, kw = ph["tasks"][t]
                    qbuf[t] = qT[:, :, qt, :]
                    bbuf[0, t, : kw * P] = np.where(
                        (np.arange(kw * P) - off * P) < klb, np.float32(0.0), NEG
                    )
                    qmbuf[:, t] = (qt * P + np.arange(P)) < qlb
            m[f"k{p}"] = kbuf
            m[f"v{p}"] = vbuf
            m[f"q{p}"] = qbuf
            m[f"bias{p}"] = bbuf
            m[f"qm{p}"] = qmbuf
        in_maps.append(m)
    return in_maps


def _run(inputs, trace=False, trace_kwargs=None):
    from concourse.bass_utils import run_bass_kernel_spmd

    queries = np.asarray(inputs["queries"], dtype=np.float32)
    keys = np.asarray(inputs["keys"], dtype=np.float32)
    values = np.asarray(inputs["values"], dtype=np.float32)
    query_lens = np.asarray(inputs["query_lens"]).astype(np.int64)
    key_lens = np.asarray(inputs["key_lens"]).astype(np.int64)
    B = queries.shape[0]
    assert B == 2 * N_CORES

    phases, skeleton, qm, km = _make_schedule(query_lens, key_lens)
    in_maps = _prep_in_maps(
        phases, qm, km, queries, keys, values, query_lens, key_lens
    )

    nc = _get_nc(skeleton)
    kwargs = {}
    if trace:
        kwargs["trace"] = True
        if trace_kwargs:
            kwargs.update(trace_kwargs)
    try:
        res = run_bass_kernel_spmd(nc, in_maps, core_ids=list(range(N_CORES)), **kwargs)
    except Exception:
        # transient device wedges usually clear on the next attempt
        import time

        time.sleep(5)
        res = run_bass_kernel_spmd(nc, in_maps, core_ids=list(range(N_CORES)), **kwargs)

    out = np.zeros((B, SEQ, D), np.float32)
    for c in range(N_CORES):
        for p, ph in enumerate(phases):
            if c >= len(ph["assign"]):
                continue
            o = res.results[c][f"out{p}"]
            b, tasks = ph["assign"][c]
            for t, qt in tasks:
                out[b, qt * P : (qt + 1) * P, :] = o[t].astype(np.float32)
    return out, res


def kernel(**inputs) -> np.ndarray:
    out, _ = _run(inputs, trace=False)
    return out


# revision 42
# speedup vs baseline: 2.7100x; 1.2745x over previous
"""Ragged masked-attention TRN2 kernel (nn_AttentionBase, B=16 Q=K=D=1024 fp32).

Sharding: data-parallel over batch, 8 cores, one SPMD program. The program
is a static schedule of per-q-tile "tasks" computed at runtime from the
actual query_lens/key_lens: rows beyond query_len and keys beyond key_len
contribute nothing to the output (softmax weight 0 / output row 0), so the
schedule only covers q-tiles < ceil(q_len/128) and per-task k-windows of
ceil(key_len/128) chunks (padded to >=2 for full-rate fp32r matmuls).

Structure: 2 phases per core (one resident K/V set each, double-buffered);
each phase has T tasks with static per-task k-window sizes taken from the
phase "profile" (elementwise max over the 8 batches assigned to the phase).
Host packs Q^T tiles / K^T / V / bias / qmask per (core, phase, task) and
scatters the bf16 task outputs back to the full fp32 output.

Numerics: scores in fp32r (PE full rate), softmax stats in fp32,
exp-weights / V / output in bf16. Measured rel err ~7e-3 vs 2e-2 gate.
"""

import sys

sys.path.insert(0, "/opt/trn_rl_repo")

from itertools import combinations

import numpy as np
import ml_dtypes

P = 128
N_CORES = 8
SEQ = 1024
D = 1024
NCH = SEQ // P  # 8 chunks of 128 along any 1024 dim
NEG = np.float32(-1e30)

_CACHE = {}


# ---------------------------------------------------------------- schedule
#
# Schedule model: each phase loads one K/V buffer of W k-chunks, laid out as
# disjoint "slots" (offset, width). A slot has a task-window profile; every
# core runs every task. A core may host at most one batch per slot (its K/V
# at the slot offset); a batch may be split across several cores' cells of
# the same slot (each cell computes a subset of its q-tiles against the
# batch's full K). Cost per core is static: sum over tasks of the window
# size. Search minimizes an estimated wall time (PE vs DMA roofline).

W_MAX = 10  # max K chunks resident per phase (SBUF budget)


def _slot_profile(members, qm, km):
    """members: {batch: ncells}. Task profile (desc): profile[t] = max km
    over members whose per-cell tile count exceeds t."""
    tiles = {b: -(-qm[b] // n) for b, n in members.items()}
    L = max(tiles.values())
    return [max(km[b] for b in members if tiles[b] > t) for t in range(L)]


NS_PER_MB = 2900.0  # DMA chain rate (344 GB/s aggregate)
UNIT_NS = 1000.0  # PE ns per (qtile x kchunk) unit at full pstate incl. overheads
TASK_NS = 350.0  # per-task pipeline bubble
ISSUE_NS = 1800.0  # DMA issue ramp before first transfer
TAIL_NS = 6000.0  # drain + last store


def _eval_state(state, qm, km):
    """Mini-sim: DMA transfers form one serial chain (K_s, V_s, per-slot Qs,
    next phase...); each slot's tasks start after its K (+V slightly before
    their tail) has landed and the PE is free."""
    units = tasks = 0
    phases = []
    for ph in state:
        W = 0
        slots = []
        for s in ph:
            if not s["m"]:
                continue
            if sum(s["m"].values()) > 8 or any(km[b] > s["w"] for b in s["m"]):
                return None
            prof = _slot_profile(s["m"], qm, km)
            units += sum(prof)
            tasks += len(prof)
            W += s["w"]
            slots.append((s["w"], sorted(prof)))
        if W > W_MAX or W == 0:
            return None
        slots.sort()
        phases.append(slots)

    def chain_wall(ordered):
        dma_t = ISSUE_NS
        pe_t = ISSUE_NS
        ramp_end = 1e18  # mid-pstate (2x cycles) until 3us continuous busy
        out_mb = 0.0
        per_dma = 680.0  # per-transfer issue/HWDGE overhead in the chain
        for slots in ordered:
            for w, prof in slots:
                dma_t += w * 0.5 * NS_PER_MB + per_dma  # K f32r
                k_done = dma_t
                dma_t += w * 0.25 * NS_PER_MB + per_dma  # V bf16
                v_done = dma_t
                for kw in prof:
                    dma_t += 0.5 * NS_PER_MB + per_dma  # Q tile f32r
                    q_done = dma_t
                    start = max(pe_t, k_done, v_done - 2000.0, q_done - 1000.0)
                    if start - pe_t > 200.0:
                        ramp_end = start + 3000.0  # stall resets the PE pstate
                    cost = kw * UNIT_NS + TASK_NS
                    if start < ramp_end:
                        cost += min(ramp_end - start, cost)  # 2x inside ramp
                    pe_t = start + cost
                    out_mb += 0.25
        return max(pe_t, dma_t + out_mb * NS_PER_MB * 0.6) + TAIL_NS

    from itertools import permutations

    best = None
    for perm in permutations(range(len(phases))):
        wall = chain_wall([phases[i] for i in perm])
        if best is None or wall < best[0]:
            best = (wall, perm)
    return best[0], units, tasks, best[1]


def _search_schedule(qm, km, iters=60000, seed=0):
    """Simulated-annealing search over slot layouts/assignments."""
    import math
    import random

    B = len(qm)
    rng = random.Random(seed)

    # init: best 8/8 partition, one slot per phase (variant-c equivalent)
    def profile_cost(group):
        T = max(qm[b] for b in group)
        return sum(max(km[b] for b in group if qm[b] > t) for t in range(T))

    best_part = None
    allb = list(range(B))
    for g0 in combinations(allb, B // 2):
        g1 = tuple(b for b in allb if b not in g0)
        c = profile_cost(g0) + profile_cost(g1)
        if best_part is None or c < best_part[0]:
            best_part = (c, g0, g1)
    _, g0, g1 = best_part
    state = [
        [{"w": max(km[b] for b in g), "m": {b: 1 for b in g}}] for g in (g0, g1)
    ]

    cur = _eval_state(state, qm, km)
    assert cur is not None
    best = (cur[0], [[{"w": s["w"], "m": dict(s["m"])} for s in ph] for ph in state])

    def copy_state(st):
        return [[{"w": s["w"], "m": dict(s["m"])} for s in ph] for ph in st]

    temp0 = 3000.0
    for it in range(iters):
        temp = temp0 * (1.0 - it / iters) + 1.0
        cand = copy_state(state)
        # pick a random batch placement
        locs = [
            (pi, si, b)
            for pi, ph in enumerate(cand)
            for si, s in enumerate(ph)
            for b in s["m"]
        ]
        pi, si, b = locs[rng.randrange(len(locs))]
        mv = rng.random()
        if mv < 0.45:
            # relocate batch to another slot (possibly new)
            n = cand[pi][si]["m"].pop(b)
            tpi = rng.randrange(len(cand))
            tph = cand[tpi]
            choices = [s for s in tph if s["w"] >= km[b]] + ["new"]
            tgt = choices[rng.randrange(len(choices))]
            if tgt == "new":
                tph.append({"w": km[b], "m": {b: n}})
            else:
                tgt["m"][b] = tgt["m"].get(b, 0) + n if False else n
        elif mv < 0.8:
            # change split factor
            n = cand[pi][si]["m"][b]
            cand[pi][si]["m"][b] = max(1, n + rng.choice([-1, 1]))
        else:
            # change slot width
            s = cand[pi][si]
            s["w"] = max(max(km[x] for x in s["m"]), s["w"] + rng.choice([-1, 1]))
        for ph in cand:
            ph[:] = [s for s in ph if s["m"]]
        if not all(ph for ph in cand):
            continue
        r = _eval_state(cand, qm, km)
        if r is None:
            continue
        if r[0] <= cur[0] or rng.random() < math.exp((cur[0] - r[0]) / temp):
            state, cur = cand, r
            if r[0] < best[0]:
                best = (r[0], copy_state(state))
    return best[1]


def _make_schedule(query_lens, key_lens):
    B = len(query_lens)
    qm = [max(1, -(-int(q) // P)) for q in query_lens]
    km = [max(2, -(-int(k) // P)) for k in key_lens]  # >=2 keeps matmul F>=256

    global W_MAX
    wmax_configs = sorted({max(w, max(km)) for w in (8, 9)})
    if "seed" in _CACHE:  # sweep override: single config
        wmax_configs = [max(W_MAX, max(km))]
        seeds = [_CACHE["seed"]]
    else:
        seeds = [0, 1, 2]
    sched_key = ("sched", tuple(qm), tuple(km))
    if sched_key in _CACHE:
        return _CACHE[sched_key]
    best = None
    for wm in wmax_configs:
        W_MAX = wm
        for sd in seeds:
            st = _search_schedule(qm, km, seed=sd)
            rr = _eval_state(st, qm, km)
            if rr is not None and (best is None or rr[0] < best[0][0]):
                best = (rr, st)
    r, state = best
    _CACHE["last_eval"] = r
    # phase order chosen by the evaluator's chain model
    perm = r[3]
    import os
    if os.environ.get("PHASE_FLIP"):
        perm = tuple(reversed(perm))
    state = [state[i] for i in perm]

    phases = []
    for ph_slots in state:
        ph_slots.sort(key=lambda s: s["w"])
        tasks = []
        slots = []
        assign = [[] for _ in range(N_CORES)]
        off = 0
        for s in ph_slots:
            prof = _slot_profile(s["m"], qm, km)
            base = len(tasks)
            # ascending task order within the slot (small windows first)
            asc = prof[::-1]
            tasks.extend((off, kw) for kw in asc)
            slots.append((off, s["w"], len(prof)))
            L = len(prof)
            core = 0
            for b, n in sorted(s["m"].items()):
                # distribute qm[b] tiles over n cells (cores)
                tiles = qm[b]
                per = -(-tiles // n)
                done = 0
                for cell in range(n):
                    cnt = min(per, tiles - done)
                    if cnt <= 0:
                        break
                    # cell uses the cnt largest tasks = last cnt in asc order
                    tlist = [
                        (base + L - cnt + j, done + j) for j in range(cnt)
                    ]
                    assign[core].append((b, off, tlist))
                    core += 1
                    done += cnt
            off += s["w"]
        phases.append({"w": off, "tasks": tasks, "slots": slots, "assign": assign})
    skeleton = tuple(
        (ph["w"], tuple(ph["tasks"]), tuple(ph["slots"])) for ph in phases
    )
    result = (phases, skeleton, qm, km)
    _CACHE[("sched", tuple(qm), tuple(km))] = result
    return result


# ---------------------------------------------------------------- program


def _build_nc(skeleton):
    import concourse.bass as bass  # noqa: F401
    import concourse.mybir as mybir
    import concourse.tile as tile
    from concourse import bacc
    from concourse.masks import make_identity

    f32 = mybir.dt.float32
    f32r = mybir.dt.float32r
    bf16 = mybir.dt.bfloat16
    X = mybir.AxisListType.X
    Exp = mybir.ActivationFunctionType.Exp

    phases = [
        {"w": w, "tasks": list(tasks), "slots": list(slots)}
        for w, tasks, slots in skeleton
    ]
    # per-parity K/V buffer widths (phase p uses buffer p % 2)
    wpar = [
        max((ph["w"] for ph in phases[par::2]), default=0) for par in range(2)
    ]
    kwmax = max(kw for ph in phases for _, kw in ph["tasks"])
    tmax = max(len(ph["tasks"]) for ph in phases)

    nc = bacc.Bacc("TRN2", target_bir_lowering=False, debug=False)

    for p, ph in enumerate(phases):
        w, T = ph["w"], len(ph["tasks"])
        ph["k_d"] = nc.dram_tensor(f"k{p}", [NCH, P, w * P], f32r, kind="ExternalInput")
        ph["v_d"] = nc.dram_tensor(f"v{p}", [w, P, D], bf16, kind="ExternalInput")
        ph["q_d"] = nc.dram_tensor(f"q{p}", [T, NCH, P, P], f32r, kind="ExternalInput")
        ph["bias_d"] = nc.dram_tensor(
            f"bias{p}", [1, T, kwmax * P], f32r, kind="ExternalInput"
        )
        ph["qm_d"] = nc.dram_tensor(f"qm{p}", [P, T], f32, kind="ExternalInput")
        ph["out_d"] = nc.dram_tensor(f"out{p}", [T, P, D], bf16, kind="ExternalOutput")

    def score_chunks(kw):
        """Split kw*128 score columns into PSUM chunks, each <=512 and
        (for fp32r full rate) >=256 columns."""
        n = kw * P
        if n <= 512:
            return [(0, n)]
        if n <= 896:
            h = (kw // 2) * P
            return [(0, h), (h, n - h)]
        return [(0, 512), (512, n - 512)]

    with tile.TileContext(nc) as tc:
        with (
            tc.tile_pool(name="const", bufs=1) as const_pool,
            tc.tile_pool(name="kv", bufs=1) as kv_pool,
            tc.tile_pool(name="qs", bufs=4) as qs_pool,
            tc.tile_pool(name="w", bufs=2) as w_pool,
            tc.tile_pool(name="wt", bufs=2) as wt_pool,
            tc.tile_pool(name="ob", bufs=2) as ob_pool,
            tc.tile_pool(name="stat", bufs=6) as stat,
            tc.tile_pool(name="ps_s", bufs=3, space="PSUM") as ps_s,
            tc.tile_pool(name="ps_t", bufs=3, space="PSUM") as ps_t,
            tc.tile_pool(name="ps_o", bufs=1, space="PSUM") as ps_o,
        ):
            identity_f32 = const_pool.tile([P, P], f32, tag="ident32")
            make_identity(nc, identity_f32)
            identity = const_pool.tile([P, P], bf16, tag="ident")
            nc.vector.tensor_copy(identity[:], identity_f32[:])
            ones_f32 = const_pool.tile([1, P], f32, tag="ones32")
            nc.gpsimd.memset(ones_f32[:], 1.0)
            ones = const_pool.tile([1, P], f32r, tag="ones")
            nc.vector.tensor_copy(ones[:], ones_f32[:])

            # per-phase-parity K/V/bias/qmask tiles (double buffer)
            kt = {}
            vt = {}
            bt = {}
            qmt = {}
            for par in range(2):
                if wpar[par] == 0:
                    continue
                kt[par] = kv_pool.tile(
                    [P, NCH, wpar[par] * P], f32r, tag=f"k{par}", name=f"k{par}"
                )
                vt[par] = kv_pool.tile(
                    [P, wpar[par], D], bf16, tag=f"v{par}", name=f"v{par}"
                )
                bt[par] = kv_pool.tile(
                    [1, tmax, kwmax * P], f32r, tag=f"b{par}", name=f"b{par}"
                )
                qmt[par] = kv_pool.tile([P, tmax], f32, tag=f"m{par}", name=f"m{par}")

            def load_misc(p):
                ph, par = phases[p], p % 2
                T = len(ph["tasks"])
                nc.gpsimd.dma_start(bt[par][:, :T], ph["bias_d"].ap())
                nc.gpsimd.dma_start(qmt[par][:, :T], ph["qm_d"].ap())

            def load_slot_k(p, off, w):
                par = p % 2
                nc.sync.dma_start(
                    kt[par][:, :, off * P : (off + w) * P],
                    phases[p]["k_d"].ap()[:, :, off * P : (off + w) * P].rearrange(
                        "d p c -> p d c"
                    ),
                )

            def load_slot_v(p, off, w):
                par = p % 2
                nc.sync.dma_start(
                    vt[par][:, off : off + w],
                    phases[p]["v_d"].ap()[off : off + w].rearrange("j p c -> p j c"),
                )

            def load_q(p, t):
                qtile = qs_pool.tile([P, NCH, P], f32r, tag="q", name=f"q{p}_{t}")
                nc.sync.dma_start(
                    qtile[:], phases[p]["q_d"].ap()[t].rearrange("d p c -> p d c")
                )
                return qtile

            stageb = {}

            def emit_stage_a(p, t, qtile):
                ph, par = phases[p], p % 2
                off, kw = ph["tasks"][t]
                chunks = score_chunks(kw)
                w_sb = w_pool.tile([P, kwmax * P], bf16, tag="w", name="w")
                nm2 = stat.tile([P, 2], f32, tag="nm2", name="nm2")
                rs = stat.tile([P, 2], f32, tag="rs", name="rs")
                pss = []
                for i, (c0, sz) in enumerate(chunks):
                    ps = ps_s.tile([P, 512], f32, tag="s", name=f"s{i}")
                    for d in range(NCH):
                        nc.tensor.matmul(
                            ps[:, :sz],
                            qtile[:, d],
                            kt[par][:, d, off * P + c0 : off * P + c0 + sz],
                            start=(d == 0),
                            stop=False,
                        )
                    nc.tensor.matmul(
                        ps[:, :sz],
                        ones[:],
                        bt[par][:, t, c0 : c0 + sz],
                        start=False,
                        stop=True,
                    )
                    nc.vector.reduce_max(nm2[:, i : i + 1], ps[:, :sz], axis=X, negate=True)
                    pss.append(ps)
                if len(chunks) == 2:
                    negmax = stat.tile([P, 1], f32, tag="negmax", name="negmax")
                    nc.vector.tensor_tensor(
                        negmax[:], nm2[:, 0:1], nm2[:, 1:2], mybir.AluOpType.min
                    )
                else:
                    negmax = nm2[:, 0:1]
                for i, (c0, sz) in enumerate(chunks):
                    nc.scalar.activation(
                        w_sb[:, c0 : c0 + sz],
                        pss[i][:, :sz],
                        Exp,
                        bias=negmax if len(chunks) == 2 else nm2[:, 0:1],
                        accum_out=rs[:, i : i + 1],
                    )
                if len(chunks) == 2:
                    rsum = stat.tile([P, 1], f32, tag="rsum", name="rsum")
                    nc.vector.tensor_tensor(
                        rsum[:], rs[:, 0:1], rs[:, 1:2], mybir.AluOpType.add
                    )
                else:
                    rsum = rs[:, 0:1]
                rcp = stat.tile([P, 1], f32, tag="rcp", name="rcp")
                nc.vector.reciprocal(rcp[:], rsum)
                scal = stat.tile([P, 1], f32, tag="scal", name="scal")
                nc.vector.tensor_tensor(
                    scal[:], rcp[:], qmt[par][:, t : t + 1], mybir.AluOpType.mult
                )
                stageb[(p, t)] = (w_sb, scal)

            def emit_stage_b(p, t, last):
                ph, par = phases[p], p % 2
                off, kw = ph["tasks"][t]
                w_sb, scal = stageb.pop((p, t))
                wts = []
                for j in range(kw):
                    pst = ps_t.tile([P, P], bf16, tag="pst", name="pst")
                    nc.tensor.transpose(
                        pst[:], w_sb[:, j * P : (j + 1) * P], identity[:]
                    )
                    wtj = wt_pool.tile([P, P], bf16, tag=f"wt{j}", name=f"wt{j}")
                    nc.any.tensor_copy(wtj[:], pst[:])
                    wts.append(wtj)
                out_sb = ob_pool.tile([P, D], bf16, tag="outsb")
                for n2 in range(2):
                    po = ps_o.tile([P, 512], f32, tag=f"o{n2}", name=f"o{n2}")
                    for j in range(kw):
                        nc.tensor.matmul(
                            po[:],
                            wts[j][:],
                            vt[par][:, off + j, n2 * 512 : (n2 + 1) * 512],
                            start=(j == 0),
                            stop=(j == kw - 1),
                        )
                    nc.any.tensor_scalar_mul(
                        out_sb[:, n2 * 512 : (n2 + 1) * 512], po[:], scal[:]
                    )
                out_eng = nc.sync if last else nc.gpsimd
                out_eng.dma_start(ph["out_d"].ap()[t], out_sb[:])

            # flat task list with cross-phase software pipeline. All input
            # DMAs go on ONE queue in need order (K_s, V_s, then the slot's
            # Q tiles), drained with a 2-task Q lookahead, so the serial DMA
            # chain delivers bytes just in time.
            flat = [(p, t) for p, ph in enumerate(phases) for t in range(len(ph["tasks"]))]
            dma_order = []
            for p, ph in enumerate(phases):
                t0 = 0
                for off, w, ntasks in ph["slots"]:
                    dma_order.append(("k", p, off, w))
                    dma_order.append(("v", p, off, w))
                    for t in range(t0, t0 + ntasks):
                        dma_order.append(("q", p, t))
                    t0 += ntasks
            qtiles = {}
            cursor = 0

            def drain_until(p, t):
                nonlocal cursor
                while cursor < len(dma_order):
                    e = dma_order[cursor]
                    cursor += 1
                    if e[0] == "k":
                        load_slot_k(e[1], e[2], e[3])
                    elif e[0] == "v":
                        load_slot_v(e[1], e[2], e[3])
                    else:
                        qtiles[(e[1], e[2])] = load_q(e[1], e[2])
                        if (e[1], e[2]) == (p, t):
                            return

            for p in range(len(phases)):
                load_misc(p)
            for i in range(min(3, len(flat))):
                drain_until(*flat[i])
            for i, (p, t) in enumerate(flat):
                emit_stage_a(p, t, qtiles.pop((p, t)))
                if i + 3 < len(flat):
                    drain_until(*flat[i + 3])
                if i >= 1:
                    pp, tt = flat[i - 1]
                    emit_stage_b(pp, tt, last=False)
            pp, tt = flat[-1]
            emit_stage_b(pp, tt, last=True)
    nc.compile()
    return nc


def _get_nc(skeleton=None):
    if skeleton is None:
        skeleton = _CACHE.get("last_skeleton")
        assert skeleton is not None, "no schedule computed yet"
    if ("nc", skeleton) not in _CACHE:
        _CACHE[("nc", skeleton)] = _build_nc(skeleton)
    _CACHE["last_skeleton"] = skeleton
    return _CACHE[("nc", skeleton)]


# ---------------------------------------------------------------- host side


def _prep_in_maps(phases, qm, km, queries, keys, values, query_lens, key_lens):
    kwmax = max(kw for ph in phases for _, kw in ph["tasks"])
    in_maps = []
    for c in range(N_CORES):
        m = {}
        for p, ph in enumerate(phases):
            w, T = ph["w"], len(ph["tasks"])
            kbuf = np.zeros((NCH, P, w * P), np.float32)
            vbuf = np.zeros((w, P, D), ml_dtypes.bfloat16)
            qbuf = np.zeros((T, NCH, P, P), np.float32)
            bbuf = np.zeros((1, T, kwmax * P), np.float32)
            qmbuf = np.zeros((P, T), np.float32)
            for b, off, tasks in ph["assign"][c]:
                kmb, qlb, klb = km[b], int(query_lens[b]), int(key_lens[b])
                kT = keys[b].T.reshape(NCH, P, SEQ)  # [d, p, k]
                kbuf[:, :, off * P : (off + kmb) * P] = kT[:, :, : kmb * P]
                vbuf[off : off + kmb] = (
                    values[b].reshape(NCH, P, D)[:kmb].astype(ml_dtypes.bfloat16)
                )
                qT = queries[b].T.reshape(NCH, P, NCH, P)  # [d, p, m, c]
                for t, qt in tasks:
                    toff, kw = ph["tasks"][t]
                    assert toff == off and kw >= kmb
                    qbuf[t] = qT[:, :, qt, :]
                    bbuf[0, t, : kw * P] = np.where(
                        np.arange(kw * P) < klb, np.float32(0.0), NEG
                    )
                    qmbuf[:, t] = (qt * P + np.arange(P)) < qlb
            m[f"k{p}"] = kbuf
            m[f"v{p}"] = vbuf
            m[f"q{p}"] = qbuf
            m[f"bias{p}"] = bbuf
            m[f"qm{p}"] = qmbuf
        in_maps.append(m)
    return in_maps


def _run(inputs, trace=False, trace_kwargs=None):
    from concourse.bass_utils import run_bass_kernel_spmd

    queries = np.asarray(inputs["queries"], dtype=np.float32)
    keys = np.asarray(inputs["keys"], dtype=np.float32)
    values = np.asarray(inputs["values"], dtype=np.float32)
    query_lens = np.asarray(inputs["query_lens"]).astype(np.int64)
    key_lens = np.asarray(inputs["key_lens"]).astype(np.int64)
    B = queries.shape[0]
    assert B == 2 * N_CORES

    phases, skeleton, qm, km = _make_schedule(query_lens, key_lens)
    in_maps = _prep_in_maps(
        phases, qm, km, queries, keys, values, query_lens, key_lens
    )

    nc = _get_nc(skeleton)
    kwargs = {}
    if trace:
        kwargs["trace"] = True
        if trace_kwargs:
            kwargs.update(trace_kwargs)
    try:
        res = run_bass_kernel_spmd(nc, in_maps, core_ids=list(range(N_CORES)), **kwargs)
    except Exception:
        # transient device wedges usually clear on the next attempt
        import time

        time.sleep(5)
        res = run_bass_kernel_spmd(nc, in_maps, core_ids=list(range(N_CORES)), **kwargs)

    out = np.zeros((B, SEQ, D), np.float32)
    for c in range(N_CORES):
        for p, ph in enumerate(phases):
            o = res.results[c][f"out{p}"]
            for b, off, tasks in ph["assign"][c]:
                for t, qt in tasks:
                    out[b, qt * P : (qt + 1) * P, :] = o[t].astype(np.float32)
    return out, res


def kernel(**inputs) -> np.ndarray:
    out, _ = _run(inputs, trace=False)
    return out
